# revision 1
# baseline (speedup 1.0000x reference)
"""Trainium2 Bass kernel for nn_DINLayer (DIN recommender forward pass).

Strategy (8 NeuronCores, SPMD, zero collectives):
  - The reference multiplies all attention scores by mask =
    (visited_goods_ids == 0), so only sequence positions s with a nonzero
    mask column contribute to x_inter. The host finds those positions
    (index preparation only); the device gathers just those v_series
    slices and computes their scores exactly, including the Dice
    batch-norm statistics. For typical inputs the mask is all-zero and
    x_inter == 0 exactly, so the whole attention branch vanishes.
  - Every core redundantly computes the full batch (the remaining work --
    4096 profile-embedding gathers + a 3-layer MLP -- is tiny, and any
    cross-core collective costs more in launch-skew waiting than the
    8x redundancy). No collectives, no stragglers; output from core 0.
  - Profile embeddings are gathered on-device via indirect DMA straight
    into the MLP input layout. Matmuls run on the PE with K-splitting;
    biases are folded as augmented ones-rows; per-channel vectors are
    host-replicated across partitions; per-row LayerNorm stats use ACT
    column bias/scale; batch-dim Dice stats use ones-vector matmuls.

Numerics: float32 throughout; softmax computed without max-subtraction
(logits are O(1) here, exp is safe and matches jax.nn.softmax to fp32
roundoff).
"""

from contextlib import ExitStack

import numpy as np

import concourse.bacc as bacc
import concourse.bass as bass
import concourse.tile as tile
from concourse import mybir
from concourse.bass_utils import run_bass_kernel_spmd
from concourse.masks import make_identity

F32 = mybir.dt.float32
I32 = mybir.dt.int32
AF = mybir.ActivationFunctionType
ALU = mybir.AluOpType
AX = mybir.AxisListType

NC = 8
B = 512
MT = B // 128         # 4 m-tiles of 128 batch rows
S = 100
D = 16
V = 160000
H1, H2 = 200, 80
CA = 36               # activation-unit hidden dim
EPS = 1e-3
XW = 176              # MLP input width: 128 profile + 48 x_inter


def _rep(v, p):
    v = np.asarray(v, np.float32).reshape(1, -1)
    return np.ascontiguousarray(np.tile(v, (p, 1)))


def _host_prep(inputs):
    feat_names = ["uid", "utag1", "utag2", "utag3", "utag4",
                  "i_goods_id", "i_shop_id", "i_cate_id"]
    ids = {k: np.asarray(inputs[k]).astype(np.int32) for k in feat_names}
    vg = np.asarray(inputs["visited_goods_ids"]).astype(np.int32)
    vs = np.asarray(inputs["visited_shop_ids"]).astype(np.int32)
    vc = np.asarray(inputs["visited_cate_ids"]).astype(np.int32)

    ss_vals = sorted(set(np.nonzero((vg == 0).any(axis=0))[0].tolist()))
    SS = len(ss_vals)

    f32 = lambda k: np.asarray(inputs[k], np.float32)
    table = np.ascontiguousarray(f32("embed_table"))

    W1 = f32("W_mlp1")
    W2m = f32("W_mlp2")
    m = {
        "table": table,
        "w1a": np.ascontiguousarray(W1[0:128]),
        "w1b": np.ascontiguousarray(
            np.concatenate([W1[128:176], f32("b_mlp1").reshape(1, -1)], 0)),
        "w2a": np.ascontiguousarray(W2m[0:128]),
        "w2b": np.ascontiguousarray(
            np.concatenate([W2m[128:200], f32("b_mlp2").reshape(1, -1)], 0)),
        "woa": np.ascontiguousarray(
            np.concatenate([f32("W_out"), f32("b_out").reshape(1, -1)], 0)),
        "g1r": _rep(f32("g_ln1"), 128), "be1r": _rep(f32("beta_ln1"), 128),
        "al1r": _rep(f32("alpha_mlp1"), 128),
        "g2r": _rep(f32("g_ln2"), 128), "be2r": _rep(f32("beta_ln2"), 128),
        "al2r": _rep(f32("alpha_mlp2"), 128),
    }

    # profile gather offsets: poff[p, mt*8 + f] = id of feature f, row mt*128+p
    poff = np.empty((128, MT * 8), np.int32)
    for mt in range(MT):
        for f, n in enumerate(feat_names):
            poff[:, mt * 8 + f] = ids[n][mt * 128:(mt + 1) * 128]
    m["poff"] = poff

    if SS > 0:
        Wact = f32("W_act1")
        Wa, Wb, Wc = Wact[0:48], Wact[48:96], Wact[96:144]
        W2 = Wact[144:].reshape(48, 48, CA)
        w2pp = np.empty((49, 48 * CA + CA), np.float32)
        w2pp[0:48, 0:48 * CA] = W2.transpose(1, 0, 2).reshape(48, 48 * CA)
        w2pp[48, 0:48 * CA] = (Wc - Wb).reshape(48 * CA)
        w2pp[0:48, 48 * CA:] = Wa + Wb
        w2pp[48, 48 * CA:] = f32("b_act1")
        m["w2pp"] = np.ascontiguousarray(w2pp)
        m["alactr"] = _rep(f32("alpha_act"), 128)
        m["waor"] = _rep(f32("W_act_out")[:, 0], 128)
        soff = np.empty((128, MT * 3 * SS), np.int32)
        vgsl = np.empty((128, MT * SS), np.int32)
        for mt in range(MT):
            sl = slice(mt * 128, (mt + 1) * 128)
            for si, s in enumerate(ss_vals):
                soff[:, mt * 3 * SS + si * 3 + 0] = vg[sl, s]
                soff[:, mt * 3 * SS + si * 3 + 1] = vs[sl, s]
                soff[:, mt * 3 * SS + si * 3 + 2] = vc[sl, s]
                vgsl[:, mt * SS + si] = vg[sl, s]
        m["soff"] = soff
        m["vgsl"] = vgsl

    bout_val = float(np.asarray(inputs["b_act_out"], np.float32).reshape(-1)[0])
    return SS, [dict(m) for _ in range(NC)], bout_val


def _build(SS, bout_val):
    nc = bacc.Bacc("TRN2", target_bir_lowering=False, debug=False,
                   num_devices=NC)

    def dram_in(name, shape, dtype=F32):
        return nc.dram_tensor(name, shape, dtype, kind="ExternalInput")

    table_d = dram_in("table", [V, D])
    poff_d = dram_in("poff", [128, MT * 8], I32)
    w1a_d = dram_in("w1a", [128, H1])
    w1b_d = dram_in("w1b", [49, H1])
    w2a_d = dram_in("w2a", [128, H2])
    w2b_d = dram_in("w2b", [73, H2])
    woa_d = dram_in("woa", [81, 2])
    g1r_d = dram_in("g1r", [128, H1])
    be1r_d = dram_in("be1r", [128, H1])
    al1r_d = dram_in("al1r", [128, H1])
    g2r_d = dram_in("g2r", [128, H2])
    be2r_d = dram_in("be2r", [128, H2])
    al2r_d = dram_in("al2r", [128, H2])
    if SS > 0:
        w2pp_d = dram_in("w2pp", [49, 48 * CA + CA])
        alact_d = dram_in("alactr", [128, CA])
        waor_d = dram_in("waor", [128, CA])
        soff_d = dram_in("soff", [128, MT * 3 * SS], I32)
        vgsl_d = dram_in("vgsl", [128, MT * SS], I32)
    out_d = nc.dram_tensor("out", [B, 2], F32, kind="ExternalOutput")

    with tile.TileContext(nc, num_cores=NC) as tc, ExitStack() as ctx:
        sb = ctx.enter_context(tc.tile_pool(name="sb", bufs=1))
        sb2 = ctx.enter_context(tc.tile_pool(name="sb2", bufs=2))
        ps = ctx.enter_context(tc.tile_pool(name="ps", bufs=2, space="PSUM"))
        ps1 = ctx.enter_context(tc.tile_pool(name="ps1", bufs=1, space="PSUM"))

        # ---- profile gathers straight into the MLP input layout ----
        poff_t = sb.tile([128, MT * 8], I32)
        nc.sync.dma_start(out=poff_t[:], in_=poff_d.ap())
        xfull = sb.tile([128, MT * XW], F32)
        for mt in range(MT):
            for f in range(8):
                nc.gpsimd.indirect_dma_start(
                    out=xfull[:, mt * XW + f * D: mt * XW + (f + 1) * D],
                    out_offset=None, in_=table_d.ap(),
                    in_offset=bass.IndirectOffsetOnAxis(
                        ap=poff_t[:, mt * 8 + f: mt * 8 + f + 1], axis=0))

        ident = sb.tile([128, 128], F32)
        make_identity(nc, ident[:])
        eps_col = sb.tile([128, 1], F32)
        nc.vector.memset(eps_col[:], EPS)
        ones_r = sb.tile([1, 128], F32)
        nc.vector.memset(ones_r[:], 1.0)
        ones_c = sb.tile([128, 1], F32)
        nc.vector.memset(ones_c[:], 1.0)

        # weight / replicated-vector loads (scalar HWDGE ring, off the sync path)
        def load(dr, shape, tag):
            t = sb.tile(shape, F32, tag=tag)
            nc.sync.dma_start(out=t[:], in_=dr.ap())
            return t
        w1a_t = load(w1a_d, [128, H1], "w1a")
        w1b_t = load(w1b_d, [49, H1], "w1b")
        w2a_t = load(w2a_d, [128, H2], "w2a")
        w2b_t = load(w2b_d, [73, H2], "w2b")
        woa_t = load(woa_d, [81, 2], "woa")
        g1r_t = load(g1r_d, [128, H1], "g1r")
        be1r_t = load(be1r_d, [128, H1], "be1r")
        al1r_t = load(al1r_d, [128, H1], "al1r")
        g2r_t = load(g2r_d, [128, H2], "g2r")
        be2r_t = load(be2r_d, [128, H2], "be2r")
        al2r_t = load(al2r_d, [128, H2], "al2r")
        omal1 = sb.tile([128, H1], F32)
        nc.vector.scalar_tensor_tensor(
            out=omal1[:], in0=al1r_t[:], scalar=-1.0, in1=ones_c[:]
            .to_broadcast([128, H1]), op0=ALU.mult, op1=ALU.add)
        omal2 = sb.tile([128, H2], F32)
        nc.vector.scalar_tensor_tensor(
            out=omal2[:], in0=al2r_t[:], scalar=-1.0, in1=ones_c[:]
            .to_broadcast([128, H2]), op0=ALU.mult, op1=ALU.add)

        bc1 = lambda t, n: t[:].rearrange("p (o n) -> p o n", o=1) \
                               .broadcast_to([128, MT, n])
        vw = lambda t, n: t[:].rearrange("p (o n) -> p o n", n=n)

        # ---- x_inter ----
        if SS == 0:
            for mt in range(MT):
                nc.vector.memset(xfull[:, mt * XW + 128:(mt + 1) * XW], 0.0)
        else:
            M36 = SS * CA
            soff_t = sb.tile([128, MT * 3 * SS], I32)
            nc.sync.dma_start(out=soff_t[:], in_=soff_d.ap())
            vgsl_t = sb.tile([128, MT * SS], I32)
            nc.sync.dma_start(out=vgsl_t[:], in_=vgsl_d.ap())
            alact_t = load(alact_d, [128, CA], "alact")
            waor_t = load(waor_d, [128, CA], "waor")
            w2pp_t = load(w2pp_d, [49, 48 * CA + CA], "w2pp")
            omal_act = sb.tile([128, CA], F32)
            nc.vector.scalar_tensor_tensor(
                out=omal_act[:], in0=alact_t[:], scalar=-1.0,
                in1=ones_c[:].to_broadcast([128, CA]),
                op0=ALU.mult, op1=ALU.add)

            # v_series slices for the full batch: sg[mt] [128, SS*48]
            sg = sb.tile([128, MT * SS * 48], F32)
            for mt in range(MT):
                for si in range(SS):
                    for f in range(3):
                        cc = (mt * SS + si) * 48 + f * D
                        nc.gpsimd.indirect_dma_start(
                            out=sg[:, cc:cc + D], out_offset=None,
                            in_=table_d.ap(),
                            in_offset=bass.IndirectOffsetOnAxis(
                                ap=soff_t[:, mt * 3 * SS + si * 3 + f:
                                          mt * 3 * SS + si * 3 + f + 1],
                                axis=0))

            # v_item^T (augmented): viT [49, 512]
            viT = sb.tile([49, B], F32)
            nc.vector.memset(viT[:], 1.0)
            for mt in range(MT):
                pvT = ps.tile([48, 128], F32, tag="t128", space="PSUM")
                nc.tensor.transpose(
                    out=pvT[:], in_=xfull[:, mt * XW + 80:mt * XW + 128],
                    identity=ident[:])
                nc.any.tensor_copy(viT[0:48, mt * 128:(mt + 1) * 128], pvT[:])

            # M_nat[mt] [128, 1764] and scores_pre
            NW = 48 * CA + CA
            spre = sb.tile([128, MT * M36], F32)
            for mt in range(MT):
                m_nat = sb2.tile([128, NW], F32, tag="mnat")
                for n0 in range(0, NW, 512):
                    n1 = min(n0 + 512, NW)
                    pM = ps1.tile([128, 512], F32, tag="bc", space="PSUM")
                    nc.tensor.matmul(
                        out=pM[:, 0:n1 - n0],
                        lhsT=viT[:, mt * 128:(mt + 1) * 128],
                        rhs=w2pp_t[:, n0:n1], start=True, stop=True)
                    nc.any.tensor_copy(m_nat[:, n0:n1], pM[:, 0:n1 - n0])
                for si in range(SS):
                    vsl = sg[:, (mt * SS + si) * 48:(mt * SS + si + 1) * 48]
                    prod = sb2.tile([128, 48 * CA], F32, tag="sprod")
                    nc.vector.tensor_tensor(
                        out=prod[:].rearrange("p (i c) -> p i c", c=CA),
                        in0=vsl.rearrange("p (i c) -> p i c", c=1)
                            .broadcast_to([128, 48, CA]),
                        in1=m_nat[:, 0:48 * CA]
                            .rearrange("p (i c) -> p i c", c=CA),
                        op=ALU.mult)
                    red = sb2.tile([128, CA], F32, tag="sred")
                    nc.vector.tensor_reduce(
                        out=red[:],
                        in_=prod[:].rearrange("p (i c) -> p c i", c=CA),
                        axis=AX.X, op=ALU.add)
                    nc.vector.tensor_tensor(
                        out=spre[:, (mt * SS + si) * CA:
                                 (mt * SS + si + 1) * CA],
                        in0=red[:], in1=m_nat[:, 48 * CA:], op=ALU.add)

            # dice-1 stats over the batch (local: full batch on this core)
            sq1 = sb.tile([128, MT * M36], F32)
            nc.vector.tensor_tensor(out=sq1[:], in0=spre[:], in1=spre[:],
                                    op=ALU.mult)
            stg = sb.tile([1, 2 * M36], F32)
            for (src, off) in ((spre, 0), (sq1, M36)):
                for n0 in range(0, M36, 512):
                    n1 = min(n0 + 512, M36)
                    pst = ps1.tile([1, 512], F32, tag="st", space="PSUM")
                    for mt in range(MT):
                        nc.tensor.matmul(
                            out=pst[:, 0:n1 - n0], lhsT=ones_c[:],
                            rhs=src[:, mt * M36 + n0:mt * M36 + n1],
                            start=(mt == 0), stop=(mt == MT - 1))
                    nc.any.tensor_copy(stg[:, off + n0:off + n1],
                                       pst[:, 0:n1 - n0])
            mu1 = sb.tile([1, M36], F32)
            nc.scalar.mul(mu1[:], stg[:, 0:M36], 1.0 / B)
            ex2 = sb.tile([1, M36], F32)
            nc.scalar.mul(ex2[:], stg[:, M36:], 1.0 / B)
            musq1 = sb.tile([1, M36], F32)
            nc.vector.tensor_tensor(out=musq1[:], in0=mu1[:], in1=mu1[:],
                                    op=ALU.mult)
            var1 = sb.tile([1, M36], F32)
            nc.vector.tensor_tensor(out=var1[:], in0=ex2[:], in1=musq1[:],
                                    op=ALU.subtract)
            sd1 = sb.tile([1, M36], F32)
            nc.scalar.activation(sd1[:], var1[:], AF.Sqrt,
                                 bias=eps_col[0:1, :], scale=1.0)
            rsq1 = sb.tile([1, M36], F32)
            nc.vector.reciprocal(rsq1[:], sd1[:])
            nmu1 = sb.tile([1, M36], F32)
            nc.vector.scalar_tensor_tensor(
                out=nmu1[:], in0=mu1[:], scalar=-1.0, in1=rsq1[:],
                op0=ALU.mult, op1=ALU.mult)
            ab1 = sb.tile([128, 2 * M36], F32)
            for (src, off) in ((rsq1, 0), (nmu1, M36)):
                for n0 in range(0, M36, 512):
                    n1 = min(n0 + 512, M36)
                    pbc = ps1.tile([128, 512], F32, tag="bc", space="PSUM")
                    nc.tensor.matmul(out=pbc[:, 0:n1 - n0], lhsT=ones_r[:],
                                     rhs=src[:, n0:n1], start=True, stop=True)
                    nc.any.tensor_copy(ab1[:, off + n0:off + n1],
                                       pbc[:, 0:n1 - n0])

            # dice-1 chain + scores + x_inter, per m-tile
            almt = lambda t: t[:].rearrange("p (o c) -> p o c", o=1) \
                                 .broadcast_to([128, SS, CA])
            for mt in range(MT):
                sl = slice(mt * M36, (mt + 1) * M36)
                xn1 = sb2.tile([128, M36], F32, tag="d1xn")
                nc.vector.tensor_tensor(out=xn1[:], in0=spre[:, sl],
                                        in1=ab1[:, 0:M36], op=ALU.mult)
                nc.vector.tensor_tensor(out=xn1[:], in0=xn1[:],
                                        in1=ab1[:, M36:], op=ALU.add)
                p1 = sb2.tile([128, M36], F32, tag="d1p")
                nc.scalar.activation(p1[:], xn1[:], AF.Sigmoid)
                f1 = sb2.tile([128, M36], F32, tag="d1f")
                v3 = lambda t: t[:].rearrange("p (s c) -> p s c", c=CA)
                nc.vector.tensor_tensor(out=v3(f1), in0=v3(p1),
                                        in1=almt(omal_act), op=ALU.mult)
                nc.vector.tensor_tensor(out=v3(f1), in0=v3(f1),
                                        in1=almt(alact_t), op=ALU.add)
                hsc = sb2.tile([128, M36], F32, tag="d1h")
                nc.vector.tensor_tensor(out=hsc[:], in0=spre[:, sl],
                                        in1=f1[:], op=ALU.mult)
                nc.vector.tensor_tensor(out=v3(hsc), in0=v3(hsc),
                                        in1=almt(waor_t), op=ALU.mult)
                sc = sb2.tile([128, SS], F32, tag="d1sc")
                nc.vector.tensor_reduce(
                    out=sc[:], in_=hsc[:].rearrange("p (s c) -> p s c", c=CA),
                    axis=AX.X, op=ALU.add)
                msk = sb2.tile([128, SS], F32, tag="d1m")
                nc.vector.tensor_scalar(
                    out=msk[:], in0=vgsl_t[:, mt * SS:(mt + 1) * SS],
                    scalar1=0, scalar2=None, op0=ALU.is_equal)
                sm = sb2.tile([128, SS], F32, tag="d1sm")
                nc.vector.scalar_tensor_tensor(
                    out=sm[:], in0=sc[:], scalar=bout_val, in1=msk[:],
                    op0=ALU.add, op1=ALU.mult)
                xin = xfull[:, mt * XW + 128:(mt + 1) * XW]
                xt = sb2.tile([128, 48], F32, tag="d1xt")
                for si in range(SS):
                    vsl = sg[:, (mt * SS + si) * 48:(mt * SS + si + 1) * 48]
                    if si == 0:
                        nc.scalar.activation(xin, vsl, AF.Copy,
                                             scale=sm[:, 0:1])
                    else:
                        nc.scalar.activation(xt[:], vsl, AF.Copy,
                                             scale=sm[:, si:si + 1])
                        nc.vector.tensor_tensor(out=xin, in0=xin, in1=xt[:],
                                                op=ALU.add)

        # ---- MLP input transposes ----
        xTa = sb.tile([128, B], F32)
        xTb = sb.tile([49, B], F32)
        nc.vector.memset(xTb[:], 1.0)
        if SS == 0:
            nc.vector.memset(xTb[0:48, :], 0.0)
        for mt in range(MT):
            pT = ps.tile([128, 128], F32, tag="t128", space="PSUM")
            nc.tensor.transpose(out=pT[:],
                                in_=xfull[:, mt * XW:mt * XW + 128],
                                identity=ident[:])
            nc.any.tensor_copy(xTa[:, mt * 128:(mt + 1) * 128], pT[:])
            if SS > 0:
                pTb = ps.tile([48, 128], F32, tag="t128", space="PSUM")
                nc.tensor.transpose(
                    out=pTb[:], in_=xfull[:, mt * XW + 128:(mt + 1) * XW],
                    identity=ident[:])
                nc.any.tensor_copy(xTb[0:48, mt * 128:(mt + 1) * 128], pTb[:])

        def dense_ln_dice(xTa_, ka, wa_t, xTb_, kb, wb_t, NH, g_t, be_t,
                          al_t, omal_t, tag):
            """Dense (K-split) + row-LN + batch-Dice, batched across MT."""
            hcat = sb.tile([128, MT * NH], F32, tag=f"{tag}_hcat")
            for mt in range(MT):
                ph = ps.tile([128, NH], F32, tag="mm", space="PSUM")
                nc.tensor.matmul(out=ph[:],
                                 lhsT=xTa_[0:ka, mt * 128:(mt + 1) * 128],
                                 rhs=wa_t[:], start=True, stop=False)
                nc.tensor.matmul(out=ph[:],
                                 lhsT=xTb_[0:kb, mt * 128:(mt + 1) * 128],
                                 rhs=wb_t[:], start=False, stop=True)
                nc.any.tensor_copy(hcat[:, mt * NH:(mt + 1) * NH], ph[:])
            # per-row LN stats, batched: [128, MT]
            ssum = sb.tile([128, MT], F32, tag=f"{tag}_ssum")
            nc.vector.tensor_reduce(out=ssum[:], in_=vw(hcat, NH),
                                    axis=AX.X, op=ALU.add)
            sq = sb.tile([128, MT * NH], F32, tag=f"{tag}_sq")
            nc.vector.tensor_tensor(out=sq[:], in0=hcat[:], in1=hcat[:],
                                    op=ALU.mult)
            ssq = sb.tile([128, MT], F32, tag=f"{tag}_ssq")
            nc.vector.tensor_reduce(out=ssq[:], in_=vw(sq, NH),
                                    axis=AX.X, op=ALU.add)
            mu = sb.tile([128, MT], F32, tag=f"{tag}_mu")
            nc.scalar.mul(mu[:], ssum[:], 1.0 / NH)
            musq = sb.tile([128, MT], F32, tag=f"{tag}_musq")
            nc.vector.tensor_tensor(out=musq[:], in0=mu[:], in1=mu[:],
                                    op=ALU.mult)
            var = sb.tile([128, MT], F32, tag=f"{tag}_var")
            nc.vector.scalar_tensor_tensor(
                out=var[:], in0=ssq[:], scalar=1.0 / NH, in1=musq[:],
                op0=ALU.mult, op1=ALU.subtract)
            sd = sb.tile([128, MT], F32, tag=f"{tag}_sd")
            nc.scalar.activation(sd[:], var[:], AF.Sqrt, bias=eps_col[:],
                                 scale=1.0)
            rsq = sb.tile([128, MT], F32, tag=f"{tag}_rsq")
            nc.vector.reciprocal(rsq[:], sd[:])
            nmu = sb.tile([128, MT], F32, tag=f"{tag}_nmu")
            nc.vector.scalar_tensor_tensor(
                out=nmu[:], in0=mu[:], scalar=-1.0, in1=rsq[:],
                op0=ALU.mult, op1=ALU.mult)
            xn = sb.tile([128, MT * NH], F32, tag=f"{tag}_xn")
            for mt in range(MT):
                nc.scalar.activation(xn[:, mt * NH:(mt + 1) * NH],
                                     hcat[:, mt * NH:(mt + 1) * NH],
                                     AF.Identity, bias=nmu[:, mt:mt + 1],
                                     scale=rsq[:, mt:mt + 1])
            ln = sb.tile([128, MT * NH], F32, tag=f"{tag}_ln")
            nc.vector.tensor_tensor(out=vw(ln, NH), in0=vw(xn, NH),
                                    in1=bc1(g_t, NH), op=ALU.mult)
            nc.vector.tensor_tensor(out=vw(ln, NH), in0=vw(ln, NH),
                                    in1=bc1(be_t, NH), op=ALU.add)
            # batch-dim dice stats via ones-matmuls
            sqln = sb.tile([128, MT * NH], F32, tag=f"{tag}_sqln")
            nc.vector.tensor_tensor(out=sqln[:], in0=ln[:], in1=ln[:],
                                    op=ALU.mult)
            pst = ps1.tile([1, 2 * NH], F32, tag="st", space="PSUM")
            for mt in range(MT):
                nc.tensor.matmul(out=pst[:, 0:NH], lhsT=ones_c[:],
                                 rhs=ln[:, mt * NH:(mt + 1) * NH],
                                 start=(mt == 0), stop=(mt == MT - 1))
            for mt in range(MT):
                nc.tensor.matmul(out=pst[:, NH:2 * NH], lhsT=ones_c[:],
                                 rhs=sqln[:, mt * NH:(mt + 1) * NH],
                                 start=(mt == 0), stop=(mt == MT - 1))
            dmu = sb.tile([1, NH], F32, tag=f"{tag}_dmu")
            nc.scalar.mul(dmu[:], pst[:, 0:NH], 1.0 / B)
            dex2 = sb.tile([1, NH], F32, tag=f"{tag}_dex2")
            nc.scalar.mul(dex2[:], pst[:, NH:2 * NH], 1.0 / B)
            dmusq = sb.tile([1, NH], F32, tag=f"{tag}_dmusq")
            nc.vector.tensor_tensor(out=dmusq[:], in0=dmu[:], in1=dmu[:],
                                    op=ALU.mult)
            dvar = sb.tile([1, NH], F32, tag=f"{tag}_dvar")
            nc.vector.tensor_tensor(out=dvar[:], in0=dex2[:], in1=dmusq[:],
                                    op=ALU.subtract)
            dsd = sb.tile([1, NH], F32, tag=f"{tag}_dsd")
            nc.scalar.activation(dsd[:], dvar[:], AF.Sqrt,
                                 bias=eps_col[0:1, :], scale=1.0)
            drsq = sb.tile([1, NH], F32, tag=f"{tag}_drsq")
            nc.vector.reciprocal(drsq[:], dsd[:])
            dnmu = sb.tile([1, NH], F32, tag=f"{tag}_dnmu")
            nc.vector.scalar_tensor_tensor(
                out=dnmu[:], in0=dmu[:], scalar=-1.0, in1=drsq[:],
                op0=ALU.mult, op1=ALU.mult)
            pbc = ps1.tile([128, 2 * NH], F32, tag="bc", space="PSUM")
            nc.tensor.matmul(out=pbc[:, 0:NH], lhsT=ones_r[:], rhs=drsq[:],
                             start=True, stop=True)
            nc.tensor.matmul(out=pbc[:, NH:2 * NH], lhsT=ones_r[:],
                             rhs=dnmu[:], start=True, stop=True)
            ab = sb.tile([128, 2 * NH], F32, tag=f"{tag}_ab")
            nc.any.tensor_copy(ab[:], pbc[:])
            # dice chain, batched across MT
            xn2 = sb.tile([128, MT * NH], F32, tag=f"{tag}_xn2")
            nc.vector.tensor_tensor(out=vw(xn2, NH), in0=vw(ln, NH),
                                    in1=bc1t(ab, 0, NH), op=ALU.mult)
            nc.vector.tensor_tensor(out=vw(xn2, NH), in0=vw(xn2, NH),
                                    in1=bc1t(ab, NH, NH), op=ALU.add)
            p = sb.tile([128, MT * NH], F32, tag=f"{tag}_p")
            nc.scalar.activation(p[:], xn2[:], AF.Sigmoid)
            fg = sb.tile([128, MT * NH], F32, tag=f"{tag}_fg")
            nc.vector.tensor_tensor(out=vw(fg, NH), in0=vw(p, NH),
                                    in1=bc1(omal_t, NH), op=ALU.mult)
            nc.vector.tensor_tensor(out=vw(fg, NH), in0=vw(fg, NH),
                                    in1=bc1(al_t, NH), op=ALU.add)
            h = sb.tile([128, MT * NH], F32, tag=f"{tag}_h")
            nc.vector.tensor_tensor(out=h[:], in0=ln[:], in1=fg[:],
                                    op=ALU.mult)
            return h

        def bc1t(t, off, n):
            return t[:, off:off + n].rearrange("p (o n) -> p o n", o=1) \
                                    .broadcast_to([128, MT, n])

        h1 = dense_ln_dice(xTa, 128, w1a_t, xTb, 49, w1b_t, H1,
                           g1r_t, be1r_t, al1r_t, omal1, "L1")

        h1Ta = sb.tile([128, B], F32)
        h1Tb = sb.tile([73, B], F32)
        nc.vector.memset(h1Tb[:], 1.0)
        for mt in range(MT):
            pT = ps.tile([128, 128], F32, tag="t128", space="PSUM")
            nc.tensor.transpose(out=pT[:],
                                in_=h1[:, mt * H1:mt * H1 + 128],
                                identity=ident[:])
            nc.any.tensor_copy(h1Ta[:, mt * 128:(mt + 1) * 128], pT[:])
            pTb = ps.tile([72, 128], F32, tag="t128", space="PSUM")
            nc.tensor.transpose(out=pTb[:],
                                in_=h1[:, mt * H1 + 128:(mt + 1) * H1],
                                identity=ident[:])
            nc.any.tensor_copy(h1Tb[0:72, mt * 128:(mt + 1) * 128], pTb[:])

        h2 = dense_ln_dice(h1Ta, 128, w2a_t, h1Tb, 73, w2b_t, H2,
                           g2r_t, be2r_t, al2r_t, omal2, "L2")

        # output layer + softmax (no max-subtraction; logits are O(1))
        h2T = sb.tile([81, B], F32)
        nc.vector.memset(h2T[:], 1.0)
        for mt in range(MT):
            pTo = ps.tile([80, 128], F32, tag="t128", space="PSUM")
            nc.tensor.transpose(out=pTo[:],
                                in_=h2[:, mt * H2:mt * H2 + 80],
                                identity=ident[:])
            nc.any.tensor_copy(h2T[0:80, mt * 128:(mt + 1) * 128], pTo[:])

        logit = sb.tile([128, MT * 2], F32)
        for mt in range(MT):
            po = ps.tile([128, 2], F32, tag="mm", space="PSUM")
            nc.tensor.matmul(out=po[:], lhsT=h2T[:, mt * 128:(mt + 1) * 128],
                             rhs=woa_t[:], start=True, stop=True)
            nc.any.tensor_copy(logit[:, mt * 2:(mt + 1) * 2], po[:])
        ex = sb.tile([128, MT * 2], F32)
        nc.scalar.activation(ex[:], logit[:], AF.Exp)
        sme = sb.tile([128, MT], F32)
        nc.vector.tensor_reduce(out=sme[:], in_=vw(ex, 2), axis=AX.X,
                                op=ALU.add)
        rcp = sb.tile([128, MT], F32)
        nc.vector.reciprocal(rcp[:], sme[:])
        osb = sb.tile([128, MT * 2], F32)
        nc.vector.tensor_tensor(
            out=vw(osb, 2), in0=vw(ex, 2),
            in1=rcp[:].rearrange("p (t o) -> p t o", o=1)
                      .broadcast_to([128, MT, 2]),
            op=ALU.mult)
        nc.sync.dma_start(
            out=out_d.ap().rearrange("(t p) c -> p t c", t=MT),
            in_=osb[:].rearrange("p (t c) -> p t c", c=2))

    nc.compile()
    return nc


def kernel(**inputs) -> np.ndarray:
    SS, in_maps, bout_val = _host_prep(inputs)
    nc = _build(SS, bout_val)
    res = run_bass_kernel_spmd(nc, in_maps, core_ids=list(range(NC)))
    return res.results[0]["out"]

